# revision 1
# baseline (speedup 1.0000x reference)
"""Trainium2 Bass kernel: batched time-domain cross-correlation.

Computes, for each of 2048 (=64x32) independent pairs (fp32):
    out[g, l] = sum_k d1[g, k + l - 301] * d2[g, k],   l in [0, 603)

Algorithm: overlap-save block correlation in a half-shift (negacyclic)
real-DFT basis, so every matmul has a *shared* stationary operand (the
transform matrices) and batches all pairs in the moving operand:

  xp = d1 zero-padded/shifted; y = d2 zero-padded.
  out[B*c + j] = sum_v corr(w_{v+c}, y_v)[j]     (j in [0, B))
    w_s = xp[B*s : B*s + 2B]  (windows, stride B, length N=2B)
    y_v = y[B*v : B*v + B]    (blocks, zero-padded to N)
  Per-block circular corr via length-N negacyclic real DFT:
    bins k: Ur[k] = sum_n u[n] cos(pi n (2k+1)/N)
            Ui[k] = -sum_n u[n] sin(pi n (2k+1)/N),  k in [0, B)
    Z = X * conj(Y):  Zr = XrYr + XiYi ; Zi = XiYr - XrYi
    z[0:B] = Minv @ [Zr; Zi]  (exact: aliasing only corrupts j > B)

Mapping: forward transforms + inverse are PE matmuls with shared
stationaries; the pointwise spectral products run on the Vector engine
with the v-sum done by segmented tensor_reduce.

Sharding: data-parallel over the 2048 pairs, 256 pairs per core, 8 cores.
"""

import math
import os
import sys

import numpy as np

if "/opt/trn_rl_repo" not in sys.path:  # harness safety; axon site usually set
    sys.path.insert(0, "/opt/trn_rl_repo")

import concourse.bacc as bacc
import concourse.bass as bass
import concourse.mybir as mybir
import concourse.tile as tile
from concourse.bass_utils import run_bass_kernel_spmd

# ---- problem constants (hardcoded per contest contract) ----
NB_PAIRS, NCH, NT = 64, 32, 3000
LAGS = 603
SHIFT = 301  # NLAG + 1
NCORES = 8
G = (NB_PAIRS * NCH) // NCORES  # 256 pairs per core

# ---- tunables ----
B = int(os.environ.get("KB", "384"))  # lag/block granularity; N = 2B
GH = int(os.environ.get("KGH", "64"))  # pairs per g-chunk (SBUF working set)
USE_F32R = os.environ.get("KF32R", "1") == "1"  # full-rate matmuls (tf32-ish)
GP_FRAC = int(os.environ.get("KGP", "3"))  # every GP_FRAC-th TT stays on DVE
DT_MM = mybir.dt.float32r if USE_F32R else mybir.dt.float32
DT_Z = mybir.dt.float32r if USE_F32R else mybir.dt.float32
DT_VE = mybir.dt.float32  # vector-engine dtype

# derived
N = 2 * B
V = math.ceil(NT / B)  # y blocks
C = math.ceil(LAGS / B)  # output lag blocks
S = V + C - 1  # x windows
# fp32r ISA: innermost counts of matmul src/dst APs must be even
SP = S + (S % 2)  # padded window count (pad windows are all-zero)
CP = C + (C % 2)  # padded out-block count
assert V % 2 == 0, "y-block count must be even for fp32r"
BS = B // 128  # 128-chunks per B
NQ = N // 128  # contraction chunks of a full window
NJ = B // 128  # 128-chunks of B (bins halves / out j groups)
NR = 2 * NJ  # psum bin groups of the spectrum
U = (SP - 1) * BS + NQ  # 128-chunks in xp (covers padded windows)
NBB = U * 128
W = (V * B) // 128  # 128-chunks in y
GC = 512 // SP if SP > 4 else 128  # g per x-fwd column group
while GH % GC:
    GC -= 1
GCY = 512 // V
while GH % GCY:
    GCY -= 1
FG = 2 * GH  # g per inverse group (f = FG*CP in [256, 512] for CP in {2,4})
assert 256 <= FG * CP <= 512 and G % FG == 0

_PE_CACHE = {}
LAST_EXEC_NS = None
LAST_TRACE = None


def _matrices():
    n = np.arange(N, dtype=np.float64)[:, None]
    k = np.arange(B, dtype=np.float64)[None, :]
    theta = np.pi * n * (2 * k + 1) / N
    ffull = np.concatenate([np.cos(theta), -np.sin(theta)], axis=1)  # [N, 2B]
    minv = np.linalg.inv(ffull.T)[:B, :]  # [B, 2B]
    return ffull.astype(np.float32), minv.astype(np.float32)


def _const_tiles():
    """FW [128, NR*NQ*128]: FW[i, ((r*NQ)+q)*128 + col] = Ffull[128q+i, 128r+col]
    (r-major so each r's blocks are one contiguous DMA piece)
    MT [128, 3*NJ*NJ*128]: for zg in {Mr, Mi, -Mi}:
        MT[i, ((zg*NJ + rh)*NJ + jg)*128 + col] = M[128jg + col, 128rh + i]
    """
    ffull, minv = _matrices()
    fw = np.zeros((128, NR * NQ * 128), dtype=np.float32)
    for q in range(NQ):
        for r in range(NR):
            fw[:, (r * NQ + q) * 128 : (r * NQ + q + 1) * 128] = ffull[
                128 * q : 128 * (q + 1), 128 * r : 128 * (r + 1)
            ]
    mr = minv[:, :B]
    mi = minv[:, B:]
    mats = [mr, mi, -mi]
    mt = np.zeros((128, 3 * NJ * NJ * 128), dtype=np.float32)
    for zg in range(3):
        for rh in range(NJ):
            for jg in range(NJ):
                blk = mats[zg][128 * jg : 128 * (jg + 1), 128 * rh : 128 * (rh + 1)]
                base = ((zg * NJ + rh) * NJ + jg) * 128
                mt[:, base : base + 128] = blk.T
    return fw, mt


def build_kernel():
    nc = bacc.Bacc(
        "TRN2",
        target_bir_lowering=False,
        debug=False,
        num_devices=NCORES,
    )

    xp_d = nc.dram_tensor("xp", [128, G, U], DT_MM, kind="ExternalInput")
    yp_d = nc.dram_tensor("yp", [128, G, W], DT_MM, kind="ExternalInput")
    fw_d = nc.dram_tensor("fw", [128, NR * NQ * 128], DT_MM, kind="ExternalInput")
    mt_d = nc.dram_tensor("mt", [128, 3 * NJ * NJ * 128], DT_Z, kind="ExternalInput")
    out_d = nc.dram_tensor("out", [128, G, NJ, C], mybir.dt.float32,
                           kind="ExternalOutput")

    with tile.TileContext(nc, trace_sim=False) as tc:
        with (
            tc.tile_pool(name="const", bufs=1) as cpool,
            tc.tile_pool(name="io", bufs=2) as iopool,
            tc.tile_pool(name="spec", bufs=2) as spool,
            tc.tile_pool(name="work", bufs=3) as wpool,
            tc.tile_pool(name="zpool", bufs=1) as zpool,
            tc.tile_pool(name="psum", bufs=1, space=bass.MemorySpace.PSUM) as ppool,
        ):
            fw_t = cpool.tile([128, NR * NQ * 128], DT_MM, tag="fw")
            mt_t = cpool.tile([128, 3 * NJ * NJ * 128], DT_Z, tag="mt")
            zr = zpool.tile([128, NJ, G, CP], DT_Z, tag="zr")
            zi = zpool.tile([128, NJ, G, 2, CP], DT_Z, tag="zi")
            if CP > C:
                nc.gpsimd.memset(zr[:, :, :, C:], 0.0)
                nc.gpsimd.memset(zi[:, :, :, :, C:], 0.0)

            tt_i = 0

            def tt_eng():
                nonlocal tt_i
                e = nc.vector if tt_i % GP_FRAC == 0 else nc.gpsimd
                tt_i += 1
                return e

            outt = iopool.tile([128, G, NJ, C], mybir.dt.float32, tag="outt", bufs=1)
            for chunk in range(G // GH):
                g0 = chunk * GH
                xin = iopool.tile([128, GH, U], DT_MM, tag="xin", bufs=3)
                yin = iopool.tile([128, GH, W], DT_MM, tag="yin", bufs=3)
                nc.sync.dma_start(xin[:], xp_d.ap()[:, g0 : g0 + GH, :])
                nc.sync.dma_start(yin[:], yp_d.ap()[:, g0 : g0 + GH, :])
                if chunk == 1:
                    # mt is first needed by the inverse after chunk 1
                    nc.sync.dma_start(mt_t[:], mt_d.ap())
                if chunk == 0:
                    # consts after the first input tiles: r-pieces in use order
                    r_order0 = [x for rh in range(NJ) for x in (rh, NJ + rh)]
                    for r in r_order0:
                        nc.sync.dma_start(
                            fw_t[:, r * NQ * 128 : (r + 1) * NQ * 128],
                            fw_d.ap()[:, r * NQ * 128 : (r + 1) * NQ * 128],
                        )

                xs = spool.tile([128, NR, GH, SP], DT_VE, tag="xs")
                ys = spool.tile([128, NR, GH, V], DT_VE, tag="ys")

                # ---- forward transforms, x and y interleaved per bin
                # group; r-order pairs (rh, NJ+rh) so PW group rh unblocks
                # after two r-iterations
                r_order = [x for rh in range(NJ) for x in (rh, NJ + rh)]
                for r in r_order:
                    for cg in range(GH // GC):
                        ps = ppool.tile([128, GC, SP], mybir.dt.float32, tag="psA",
                                        bufs=4)
                        for q in range(NQ):
                            lhsT = fw_t[:, (r * NQ + q) * 128 : (r * NQ + q + 1) * 128]
                            rhs = xin[
                                :,
                                cg * GC : (cg + 1) * GC,
                                q : q + BS * (SP - 1) + 1 : BS,
                            ]
                            nc.tensor.matmul(
                                ps[:], lhsT, rhs, start=(q == 0), stop=(q == NQ - 1)
                            )
                        nc.scalar.copy(
                            out=xs[:, r, cg * GC : (cg + 1) * GC, :], in_=ps[:]
                        )
                    for cg in range(GH // GCY):
                        ps = ppool.tile([128, GCY, V], mybir.dt.float32, tag="psB",
                                        bufs=2)
                        for q in range(NJ):
                            lhsT = fw_t[:, (r * NQ + q) * 128 : (r * NQ + q + 1) * 128]
                            rhs = yin[
                                :,
                                cg * GCY : (cg + 1) * GCY,
                                q : q + BS * (V - 1) + 1 : BS,
                            ]
                            nc.tensor.matmul(
                                ps[:], lhsT, rhs, start=(q == 0), stop=(q == NJ - 1)
                            )
                        nc.scalar.copy(
                            out=ys[:, r, cg * GCY : (cg + 1) * GCY, :], in_=ps[:]
                        )

                # ---- pointwise products + v-sum (DVE + GpSimd) ----
                for c in range(C):
                    for rh in range(NJ):
                        pr = wpool.tile([128, GH, 2, V], DT_VE, tag="pr", bufs=4)
                        tt_eng().tensor_mul(
                            pr[:, :, 0, :],
                            xs[:, rh, :, c : c + V],
                            ys[:, rh, :, :],
                        )
                        tt_eng().tensor_mul(
                            pr[:, :, 1, :],
                            xs[:, NJ + rh, :, c : c + V],
                            ys[:, NJ + rh, :, :],
                        )
                        with nc.allow_low_precision(
                            "float32r output is 4-byte fp32 bits"
                        ):
                            nc.vector.tensor_reduce(
                                zr[:, rh, g0 : g0 + GH, c],
                                pr[:],
                                axis=mybir.AxisListType.XY,
                                op=mybir.AluOpType.add,
                            )
                        pr2 = wpool.tile([128, GH, 2, V], DT_VE, tag="pr", bufs=4)
                        tt_eng().tensor_mul(
                            pr2[:, :, 0, :],
                            xs[:, NJ + rh, :, c : c + V],
                            ys[:, rh, :, :],
                        )
                        tt_eng().tensor_mul(
                            pr2[:, :, 1, :],
                            xs[:, rh, :, c : c + V],
                            ys[:, NJ + rh, :, :],
                        )
                        with nc.allow_low_precision(
                            "float32r output is 4-byte fp32 bits"
                        ):
                            nc.vector.tensor_reduce(
                                zi[:, rh, g0 : g0 + GH, :, c],
                                pr2[:],
                                axis=mybir.AxisListType.X,
                                op=mybir.AluOpType.add,
                            )

                # ---- inverse transform for each completed pair-group ----
                if (chunk + 1) % (FG // GH) == 0:
                    fgi = chunk // (FG // GH)
                    gsl = slice(fgi * FG, (fgi + 1) * FG)
                    for jg in range(NJ):
                        ps = ppool.tile([128, FG, CP], mybir.dt.float32, tag="psC",
                                        bufs=2)
                        nmm = 3 * NJ
                        i = 0
                        for rh in range(NJ):
                            srcs = (
                                (0, zr[:, rh, gsl, :]),
                                (1, zi[:, rh, gsl, 0, :]),
                                (2, zi[:, rh, gsl, 1, :]),
                            )
                            for zg, rhs in srcs:
                                lhsT = mt_t[
                                    :,
                                    ((zg * NJ + rh) * NJ + jg) * 128 :
                                    ((zg * NJ + rh) * NJ + jg + 1) * 128,
                                ]
                                nc.tensor.matmul(
                                    ps[:], lhsT, rhs,
                                    start=(i == 0), stop=(i == nmm - 1),
                                )
                                i += 1
                        nc.scalar.copy(out=outt[:, gsl, jg, :], in_=ps[:, :, :C])

            nc.sync.dma_start(out_d.ap()[:], outt[:])

    nc.compile()
    return nc


def _prep_core_inputs(d1f, d2f, fw, mt, core):
    """d1f/d2f: [2048, 3000] fp32. Returns the in_map for `core`."""
    sl = slice(core * G, (core + 1) * G)
    x = d1f[sl]
    y = d2f[sl]
    xp = np.zeros((G, NBB), dtype=np.float32)
    xp[:, SHIFT : SHIFT + NT] = x
    yp = np.zeros((G, V * B), dtype=np.float32)
    yp[:, :NT] = y
    # device layouts: xpT[p, g, u] = xp[g, 128u + p]
    xpT = np.ascontiguousarray(xp.reshape(G, U, 128).transpose(2, 0, 1))
    ypT = np.ascontiguousarray(yp.reshape(G, W, 128).transpose(2, 0, 1))
    return {"xp": xpT, "yp": ypT, "fw": fw, "mt": mt}


def kernel(data1: np.ndarray, data2: np.ndarray) -> np.ndarray:
    import time

    d1f = np.ascontiguousarray(data1, dtype=np.float32).reshape(-1, NT)
    d2f = np.ascontiguousarray(data2, dtype=np.float32).reshape(-1, NT)
    fw, mt = _const_tiles()

    t0 = time.time()
    if "nc" not in _PE_CACHE:
        _PE_CACHE["nc"] = build_kernel()
    nc = _PE_CACHE["nc"]
    print(f"[kernel] build+compile {time.time() - t0:.1f}s", file=sys.stderr,
          flush=True)

    in_maps = [_prep_core_inputs(d1f, d2f, fw, mt, i) for i in range(NCORES)]
    t0 = time.time()
    res = run_bass_kernel_spmd(nc, in_maps, core_ids=list(range(NCORES)))
    print(f"[kernel] spmd run {time.time() - t0:.1f}s", file=sys.stderr, flush=True)
    global LAST_EXEC_NS, LAST_TRACE
    LAST_EXEC_NS = res.exec_time_ns
    LAST_TRACE = res.instructions_and_trace
    if res.exec_time_ns is not None:
        print(f"[kernel] HW exec {res.exec_time_ns} ns", file=sys.stderr, flush=True)

    outs = []
    for i in range(NCORES):
        o = res.results[i]["out"]  # [128, G, NJ, C]
        # out[g, B*c + 128*jg + p] = o[p, g, jg, c]
        full = o.transpose(1, 3, 2, 0).reshape(G, C * B)
        outs.append(full[:, :LAGS])
    return np.concatenate(outs, axis=0).reshape(NB_PAIRS, NCH, LAGS)



# revision 3
# speedup vs baseline: 1.1569x; 1.1569x over previous
"""Trainium2 Bass kernel: batched time-domain cross-correlation.

Computes, for each of 2048 (=64x32) independent pairs (fp32):
    out[g, l] = sum_k d1[g, k + l - 301] * d2[g, k],   l in [0, 603)

Algorithm: overlap-save block correlation in a half-shift (negacyclic)
real-DFT basis; every matmul has a shared stationary operand (the
transform matrices) and batches all pairs in the moving operand:

  xp = d1 zero-padded/shifted; y = d2 zero-padded.
  out[B*c + j] = sum_v corr(w_{v+c}, y_v)[j]     (j in [0, B))
    w_s = xp[B*s : B*s + 2B]  (windows, stride B, length N=2B)
    y_v = y[B*v : B*v + B]    (blocks, zero-padded to N)
  Per-block circular corr via length-N negacyclic real DFT:
    bins k: Ur[k] = sum_n u[n] cos(pi n (2k+1)/N)
            Ui[k] = -sum_n u[n] sin(pi n (2k+1)/N),  k in [0, B)
    Z = X * conj(Y):  Zr = XrYr + XiYi ; Zia = XiYr ; Zib = XrYi
    z[0:B] = Mr@Zr + Mi@Zia - Mi@Zib  (exact: aliasing only corrupts j > B)

v2 layout: bf16 throughout (fp32 PSUM/output). Spectra stored with the
pair index g innermost ([128, NR, s, GH]); the v-sum of spectral
products runs as a pairwise add-tree on DVE/GpSimd (bf16 2x mode)
instead of tensor_reduce, with ops split across the two engines by a
greedy cost balancer.

Sharding: data-parallel over the 2048 pairs, 256 pairs per core, 8 cores.
"""

import sys

import numpy as np

if "/opt/trn_rl_repo" not in sys.path:  # harness safety; axon site usually set
    sys.path.insert(0, "/opt/trn_rl_repo")

import ml_dtypes

import concourse.bacc as bacc
import concourse.bass as bass
import concourse.mybir as mybir
import concourse.tile as tile
from concourse.bass_utils import run_bass_kernel_spmd

# ---- problem constants (hardcoded per contest contract) ----
NB_PAIRS, NCH, NT = 64, 32, 3000
LAGS = 603
SHIFT = 301  # NLAG + 1
NCORES = 8
G = (NB_PAIRS * NCH) // NCORES  # 256 pairs per core

B = 384  # lag/block granularity; N = 2B
N = 2 * B
V = 8  # ceil(NT / B) y blocks
C = 2  # ceil(LAGS / B) output lag blocks
S = V + C - 1  # 9 x windows
BS = B // 128  # 3 chunks per B
NQ = N // 128  # 6 contraction chunks of a window
NJ = B // 128  # 3 bins-half / out-j 128-groups
NR = 2 * NJ  # 6 spectral 128-groups
U = (S - 1) * BS + NQ  # 30 128-chunks in xp
W = V * BS  # 24 128-chunks in y
GH = 64  # pairs per chunk (4 chunks)
GC = 32  # x-fwd pair group (moving S*GC = 288)
FG = 2 * GH  # pairs per inverse group (every 2 chunks)

# trailing zero chunks of xp: nonzero taps end at SHIFT+NT=3301 -> chunk 25.
# window s uses chunk 3s+q; window s contributes at position q iff 3s+q <= 25
SMAX = [min(S, (25 - q) // BS + 1) for q in range(NQ)]
QORDER = [1, 2, 3, 4, 5, 0]  # first/last emitted cover the full region

DT = mybir.dt.bfloat16
F32 = mybir.dt.float32

_PE_CACHE = {}
LAST_EXEC_NS = None
LAST_TRACE = None

# engine-balance cost constants (ns)
_DVE_PER_ELEM = 0.52
_DVE_FIXED = 85.0
_POOL_PER_ELEM = 1.98
_POOL_FIXED = 130.0


class _Balancer:
    def __init__(self, nc):
        self.nc = nc
        self.tv = 0.0
        self.tg = 0.0

    def eng(self, elems):
        cv = elems * _DVE_PER_ELEM + _DVE_FIXED
        cg = elems * _POOL_PER_ELEM + _POOL_FIXED
        if max(self.tv + cv, self.tg) <= max(self.tv, self.tg + cg):
            self.tv += cv
            return self.nc.vector
        self.tg += cg
        return self.nc.gpsimd


def _matrices():
    n = np.arange(N, dtype=np.float64)[:, None]
    k = np.arange(B, dtype=np.float64)[None, :]
    theta = np.pi * n * (2 * k + 1) / N
    ffull = np.concatenate([np.cos(theta), -np.sin(theta)], axis=1)  # [N, 2B]
    minv = np.linalg.inv(ffull.T)[:B, :]  # [B, 2B]
    return ffull.astype(np.float32), minv.astype(np.float32)


def _const_tiles():
    """FW [128, NR*NQ*128]: FW[i, ((r*NQ)+q)*128 + col] = Ffull[128q+i, 128r+col]
    MT [128, 3*NJ*NJ*128]: for zg in {Mr, Mi, -Mi}:
        MT[i, ((zg*NJ + rh)*NJ + jg)*128 + col] = M[128jg + col, 128rh + i]
    """
    ffull, minv = _matrices()
    fw = np.zeros((128, NR * NQ * 128), dtype=np.float32)
    for q in range(NQ):
        for r in range(NR):
            fw[:, (r * NQ + q) * 128 : (r * NQ + q + 1) * 128] = ffull[
                128 * q : 128 * (q + 1), 128 * r : 128 * (r + 1)
            ]
    mr = minv[:, :B]
    mi = minv[:, B:]
    mats = [mr, mi, -mi]
    mt = np.zeros((128, 3 * NJ * NJ * 128), dtype=np.float32)
    for zg in range(3):
        for rh in range(NJ):
            for jg in range(NJ):
                blk = mats[zg][128 * jg : 128 * (jg + 1), 128 * rh : 128 * (rh + 1)]
                base = ((zg * NJ + rh) * NJ + jg) * 128
                mt[:, base : base + 128] = blk.T
    bf = ml_dtypes.bfloat16
    return fw.astype(bf), mt.astype(bf)


def build_kernel():
    nc = bacc.Bacc(
        "TRN2",
        target_bir_lowering=False,
        debug=False,
        num_devices=NCORES,
    )

    xp_d = nc.dram_tensor("xp", [128, G, U], DT, kind="ExternalInput")
    yp_d = nc.dram_tensor("yp", [128, G, W], DT, kind="ExternalInput")
    fw_d = nc.dram_tensor("fw", [128, NR * NQ * 128], DT, kind="ExternalInput")
    mt_d = nc.dram_tensor("mt", [128, 3 * NJ * NJ * 128], DT, kind="ExternalInput")
    out_d = nc.dram_tensor("out", [128, NJ, C, G], F32, kind="ExternalOutput")

    with tile.TileContext(nc, trace_sim=False) as tc:
        with (
            tc.tile_pool(name="const", bufs=1) as cpool,
            tc.tile_pool(name="io", bufs=2) as iopool,
            tc.tile_pool(name="spec", bufs=2) as spool,
            tc.tile_pool(name="work", bufs=2) as wpool,
            tc.tile_pool(name="zpool", bufs=2) as zpool,
            tc.tile_pool(name="psum", bufs=1, space=bass.MemorySpace.PSUM) as ppool,
        ):
            fw_t = cpool.tile([128, NR * NQ * 128], DT, tag="fw")
            mt_t = cpool.tile([128, 3 * NJ * NJ * 128], DT, tag="mt")
            bal = _Balancer(nc)

            Z = None
            for chunk in range(G // GH):
                g0 = chunk * GH
                par = chunk % 2
                xin = iopool.tile([128, GH, U], DT, tag="xin", bufs=3)
                yin = iopool.tile([128, GH, W], DT, tag="yin", bufs=3)
                nc.sync.dma_start(xin[:], xp_d.ap()[:, g0 : g0 + GH, :])
                nc.sync.dma_start(yin[:], yp_d.ap()[:, g0 : g0 + GH, :])
                if chunk == 0:
                    # consts after the first input tiles: r-pieces in use order
                    for r in range(NR):
                        nc.sync.dma_start(
                            fw_t[:, r * NQ * 128 : (r + 1) * NQ * 128],
                            fw_d.ap()[:, r * NQ * 128 : (r + 1) * NQ * 128],
                        )
                if chunk == 1:
                    # mt is first needed by the inverse at the end of chunk 1
                    nc.sync.dma_start(mt_t[:], mt_d.ap())

                xs = spool.tile([128, NR, S, GH], DT, tag="xs")
                ys = spool.tile([128, NR, V, GH], DT, tag="ys")

                # ---- forward transforms (PE), spectra with g innermost ----
                for r in range(NR):
                    for cg in range(GH // GC):
                        ps = ppool.tile([128, S, GC], F32, tag="psA", bufs=4)
                        for qi, q in enumerate(QORDER):
                            sm = SMAX[q]
                            lhsT = fw_t[
                                :, (r * NQ + q) * 128 : (r * NQ + q + 1) * 128
                            ]
                            rhs = xin[
                                :,
                                cg * GC : (cg + 1) * GC,
                                q : q + BS * (sm - 1) + 1 : BS,
                            ].rearrange("p g s -> p s g")
                            nc.tensor.matmul(
                                ps[:, 0:sm, :],
                                lhsT,
                                rhs,
                                start=(qi == 0),
                                stop=(qi == NQ - 1),
                            )
                        nc.scalar.copy(
                            out=xs[:, r, :, cg * GC : (cg + 1) * GC], in_=ps[:]
                        )
                    psy = ppool.tile([128, V, GH], F32, tag="psB", bufs=2)
                    for q in range(NJ):
                        lhsT = fw_t[:, (r * NQ + q) * 128 : (r * NQ + q + 1) * 128]
                        rhs = yin[:, :, q : q + BS * (V - 1) + 1 : BS].rearrange(
                            "p g v -> p v g"
                        )
                        nc.tensor.matmul(
                            psy[:], lhsT, rhs, start=(q == 0), stop=(q == NJ - 1)
                        )
                    nc.scalar.copy(out=ys[:, r], in_=psy[:])

                # ---- pointwise products + v-sum add-tree (DVE/GpSimd) ----
                if par == 0:
                    Z = zpool.tile([128, 3 * NJ, C, FG], DT, tag="Z")
                gsl = slice(par * GH, (par + 1) * GH)
                for c in range(C):
                    pr1 = wpool.tile([128, NR, V, GH], DT, tag="pr1", bufs=2)
                    pr2 = wpool.tile([128, NR, V, GH], DT, tag="pr2", bufs=2)
                    m = NR * V * GH // 2
                    bal.eng(m).tensor_mul(
                        pr1[:, 0:NJ], xs[:, 0:NJ, c : c + V, :], ys[:, 0:NJ]
                    )
                    bal.eng(m).tensor_mul(
                        pr1[:, NJ:NR], xs[:, NJ:NR, c : c + V, :], ys[:, NJ:NR]
                    )
                    bal.eng(m).tensor_mul(
                        pr2[:, 0:NJ], xs[:, NJ:NR, c : c + V, :], ys[:, 0:NJ]
                    )
                    bal.eng(m).tensor_mul(
                        pr2[:, NJ:NR], xs[:, 0:NJ, c : c + V, :], ys[:, NJ:NR]
                    )
                    t1 = wpool.tile([128, NR, V // 2, GH], DT, tag="t1", bufs=2)
                    bal.eng(NR * (V // 2) * GH).tensor_add(
                        t1[:], pr1[:, :, 0:V:2, :], pr1[:, :, 1:V:2, :]
                    )
                    t2 = wpool.tile([128, NR, V // 4, GH], DT, tag="t2", bufs=2)
                    bal.eng(NR * (V // 4) * GH).tensor_add(
                        t2[:], t1[:, :, 0 : V // 2 : 2, :], t1[:, :, 1 : V // 2 : 2, :]
                    )
                    t3 = wpool.tile([128, NR, GH], DT, tag="t3", bufs=2)
                    bal.eng(NR * GH).tensor_add(
                        t3[:], t2[:, :, 0, :], t2[:, :, 1, :]
                    )
                    bal.eng(NJ * GH).tensor_add(
                        Z[:, 0:NJ, c, gsl], t3[:, 0:NJ], t3[:, NJ:NR]
                    )
                    u1 = wpool.tile([128, NR, V // 2, GH], DT, tag="u1", bufs=2)
                    bal.eng(NR * (V // 2) * GH).tensor_add(
                        u1[:], pr2[:, :, 0:V:2, :], pr2[:, :, 1:V:2, :]
                    )
                    u2 = wpool.tile([128, NR, V // 4, GH], DT, tag="u2", bufs=2)
                    bal.eng(NR * (V // 4) * GH).tensor_add(
                        u2[:], u1[:, :, 0 : V // 2 : 2, :], u1[:, :, 1 : V // 2 : 2, :]
                    )
                    bal.eng(NR * GH).tensor_add(
                        Z[:, NJ : 3 * NJ, c, gsl], u2[:, :, 0, :], u2[:, :, 1, :]
                    )

                # ---- inverse transform for each completed pair-group ----
                if par == 1:
                    fgi = chunk // 2
                    outt = iopool.tile([128, NJ, C, FG], F32, tag="outt", bufs=2)
                    for jg in range(NJ):
                        ps = ppool.tile([128, C, FG], F32, tag="psC", bufs=2)
                        i = 0
                        for zg in range(3):
                            for rh in range(NJ):
                                lhsT = mt_t[
                                    :,
                                    ((zg * NJ + rh) * NJ + jg) * 128 :
                                    ((zg * NJ + rh) * NJ + jg + 1) * 128,
                                ]
                                nc.tensor.matmul(
                                    ps[:],
                                    lhsT,
                                    Z[:, zg * NJ + rh, :, :],
                                    start=(i == 0),
                                    stop=(i == 3 * NJ - 1),
                                )
                                i += 1
                        nc.scalar.copy(out=outt[:, jg], in_=ps[:])
                    nc.sync.dma_start(
                        out_d.ap()[:, :, :, fgi * FG : (fgi + 1) * FG], outt[:]
                    )

    nc.compile()
    return nc


def _prep_core_inputs(d1f, d2f, fw, mt, core):
    """d1f/d2f: [2048, 3000] fp32. Returns the in_map for `core`."""
    bf = ml_dtypes.bfloat16
    sl = slice(core * G, (core + 1) * G)
    xp = np.zeros((G, U * 128), dtype=np.float32)
    xp[:, SHIFT : SHIFT + NT] = d1f[sl]
    yp = np.zeros((G, W * 128), dtype=np.float32)
    yp[:, :NT] = d2f[sl]
    # device layouts: xpT[p, g, u] = xp[g, 128u + p]
    xpT = np.ascontiguousarray(xp.reshape(G, U, 128).transpose(2, 0, 1)).astype(bf)
    ypT = np.ascontiguousarray(yp.reshape(G, W, 128).transpose(2, 0, 1)).astype(bf)
    return {"xp": xpT, "yp": ypT, "fw": fw, "mt": mt}


def kernel(data1: np.ndarray, data2: np.ndarray) -> np.ndarray:
    import time

    d1f = np.ascontiguousarray(data1, dtype=np.float32).reshape(-1, NT)
    d2f = np.ascontiguousarray(data2, dtype=np.float32).reshape(-1, NT)
    fw, mt = _const_tiles()

    t0 = time.time()
    if "nc" not in _PE_CACHE:
        _PE_CACHE["nc"] = build_kernel()
    nc = _PE_CACHE["nc"]
    print(f"[kernel] build+compile {time.time() - t0:.1f}s", file=sys.stderr,
          flush=True)

    in_maps = [_prep_core_inputs(d1f, d2f, fw, mt, i) for i in range(NCORES)]
    t0 = time.time()
    res = run_bass_kernel_spmd(nc, in_maps, core_ids=list(range(NCORES)))
    print(f"[kernel] spmd run {time.time() - t0:.1f}s", file=sys.stderr, flush=True)
    global LAST_EXEC_NS, LAST_TRACE
    LAST_EXEC_NS = res.exec_time_ns
    LAST_TRACE = res.instructions_and_trace
    if res.exec_time_ns is not None:
        print(f"[kernel] HW exec {res.exec_time_ns} ns", file=sys.stderr, flush=True)

    outs = []
    for i in range(NCORES):
        o = np.asarray(res.results[i]["out"], dtype=np.float32)  # [128, NJ, C, G]
        # out[g, B*c + 128*jg + p] = o[p, jg, c, g]
        full = o.transpose(3, 2, 1, 0).reshape(G, C * B)
        outs.append(full[:, :LAGS])
    return np.concatenate(outs, axis=0).reshape(NB_PAIRS, NCH, LAGS)


# revision 19
# speedup vs baseline: 1.4095x; 1.2184x over previous
"""Trainium2 Bass kernel: batched time-domain cross-correlation.

Computes, for each of 2048 (=64x32) independent pairs (fp32):
    out[g, l] = sum_k d1[g, k + l - 301] * d2[g, k],   l in [0, 603)

Algorithm: overlap-save block correlation in a half-shift (negacyclic)
real-DFT basis; every matmul has a shared stationary operand (the
transform matrices) and batches all pairs in the moving operand:

  xp = d1 zero-padded/shifted; y = d2 zero-padded.
  out[B*c + j] = sum_v corr(w_{v+c}, y_v)[j]     (j in [0, B))
    w_s = xp[B*s : B*s + 2B]  (windows, stride B, length N=2B)
    y_v = y[B*v : B*v + B]    (blocks, zero-padded to N)
  Per-block circular corr via length-N negacyclic real DFT:
    bins k: Ur[k] = sum_n u[n] cos(pi n (2k+1)/N)
            Ui[k] = -sum_n u[n] sin(pi n (2k+1)/N),  k in [0, B)
    Z = X * conj(Y):  Zr = XrYr + XiYi ; Zia = XiYr ; Zib = XrYi
    z[0:B] = Mr@Zr + Mi@Zia - Mi@Zib  (exact: aliasing only corrupts j > B)

v2 layout: bf16 throughout (fp32 PSUM/output). Spectra stored with the
pair index g innermost ([128, NR, s, GH]); the v-sum of spectral
products runs as a pairwise add-tree on DVE/GpSimd (bf16 2x mode)
instead of tensor_reduce, with ops split across the two engines by a
greedy cost balancer.

Sharding: data-parallel over the 2048 pairs, 256 pairs per core, 8 cores.
"""

import sys

import numpy as np

if "/opt/trn_rl_repo" not in sys.path:  # harness safety; axon site usually set
    sys.path.insert(0, "/opt/trn_rl_repo")

import ml_dtypes

import concourse.bacc as bacc
import concourse.bass as bass
import concourse.mybir as mybir
import concourse.tile as tile
from concourse.bass_utils import run_bass_kernel_spmd

# ---- problem constants (hardcoded per contest contract) ----
NB_PAIRS, NCH, NT = 64, 32, 3000
LAGS = 603
SHIFT = 301  # NLAG + 1
NCORES = 8
G = (NB_PAIRS * NCH) // NCORES  # 256 pairs per core

B = 384  # lag/block granularity; N = 2B
N = 2 * B
V = 8  # ceil(NT / B) y blocks
C = 2  # ceil(LAGS / B) output lag blocks
S = V + C - 1  # 9 x windows
BS = B // 128  # 3 chunks per B
NQ = N // 128  # 6 contraction chunks of a window
NJ = B // 128  # 3 bins-half / out-j 128-groups
NR = 2 * NJ  # 6 spectral 128-groups
U = (S - 1) * BS + NQ  # 30 128-chunks in xp
W = V * BS  # 24 128-chunks in y
GH = 64  # pairs per chunk (4 chunks)
GC = 32  # x-fwd pair group (moving S*GC = 288)
# tapered chunk sizes: small first chunk (fast pipeline fill) and small
# last chunk + its own inverse group (fast drain)
CHUNKS = [(0, 32), (32, 64), (96, 64), (160, 64), (224, 32)]

# trailing zero chunks of xp: nonzero taps end at SHIFT+NT=3301 -> chunk 25.
# window s uses chunk 3s+q; window s contributes at position q iff 3s+q <= 25
SMAX = [min(S, (25 - q) // BS + 1) for q in range(NQ)]
QORDER = [1, 2, 3, 4, 5, 0]  # first/last emitted cover the full region

DT = mybir.dt.bfloat16
F32 = mybir.dt.float32

_PE_CACHE = {}
LAST_EXEC_NS = None
LAST_TRACE = None

# engine-balance cost constants (ns)
_DVE_PER_ELEM = 0.52
_DVE_FIXED = 85.0
_POOL_PER_ELEM = 1.98
_POOL_FIXED = 130.0


class _Balancer:
    def __init__(self, nc):
        self.nc = nc
        self.tv = 0.0
        self.tg = 0.0

    def eng(self, elems):
        cv = elems * _DVE_PER_ELEM + _DVE_FIXED
        cg = elems * _POOL_PER_ELEM + _POOL_FIXED
        if max(self.tv + cv, self.tg) <= max(self.tv, self.tg + cg):
            self.tv += cv
            return self.nc.vector
        self.tg += cg
        return self.nc.gpsimd

    def dve(self, elems):
        self.tv += elems * _DVE_PER_ELEM + _DVE_FIXED
        return self.nc.vector


def _matrices():
    n = np.arange(N, dtype=np.float64)[:, None]
    k = np.arange(B, dtype=np.float64)[None, :]
    theta = np.pi * n * (2 * k + 1) / N
    ffull = np.concatenate([np.cos(theta), -np.sin(theta)], axis=1)  # [N, 2B]
    minv = np.linalg.inv(ffull.T)[:B, :]  # [B, 2B]
    return ffull.astype(np.float32), minv.astype(np.float32)


def _const_tiles():
    """FW [128, NR*NQ*128]: FW[i, ((r*NQ)+q)*128 + col] = Ffull[128q+i, 128r+col]
    MT [128, 3*NJ*NJ*128]: for zg in {Mr, Mi, -Mi}:
        MT[i, ((zg*NJ + rh)*NJ + jg)*128 + col] = M[128jg + col, 128rh + i]
    """
    ffull, minv = _matrices()
    fw = np.zeros((128, NR * NQ * 128), dtype=np.float32)
    for q in range(NQ):
        for r in range(NR):
            fw[:, (r * NQ + q) * 128 : (r * NQ + q + 1) * 128] = ffull[
                128 * q : 128 * (q + 1), 128 * r : 128 * (r + 1)
            ]
    mr = minv[:, :B]
    mi = minv[:, B:]
    mats = [mr, mi, -mi]
    mt = np.zeros((128, 3 * NJ * NJ * 128), dtype=np.float32)
    for zg in range(3):
        for rh in range(NJ):
            for jg in range(NJ):
                blk = mats[zg][128 * jg : 128 * (jg + 1), 128 * rh : 128 * (rh + 1)]
                base = ((zg * NJ + rh) * NJ + jg) * 128
                mt[:, base : base + 128] = blk.T
    bf = ml_dtypes.bfloat16
    return fw.astype(bf), mt.astype(bf)


def build_kernel():
    nc = bacc.Bacc(
        "TRN2",
        target_bir_lowering=False,
        debug=False,
        num_devices=NCORES,
    )

    xp_d = nc.dram_tensor("xp", [128, G, U], DT, kind="ExternalInput")
    yp_d = nc.dram_tensor("yp", [128, G, W], DT, kind="ExternalInput")
    fw_d = nc.dram_tensor("fw", [128, NR * NQ * 128], DT, kind="ExternalInput")
    mt_d = nc.dram_tensor("mt", [128, 3 * NJ * NJ * 128], DT, kind="ExternalInput")
    out_d = nc.dram_tensor("out", [128, NJ, C, G], F32, kind="ExternalOutput")

    with tile.TileContext(nc, trace_sim=False) as tc:
        with (
            tc.tile_pool(name="const", bufs=1) as cpool,
            tc.tile_pool(name="io", bufs=2) as iopool,
            tc.tile_pool(name="spec", bufs=2) as spool,
            tc.tile_pool(name="work", bufs=2) as wpool,
            tc.tile_pool(name="zpool", bufs=2) as zpool,
            tc.tile_pool(name="psum", bufs=1, space=bass.MemorySpace.PSUM) as ppool,
        ):
            fw_t = cpool.tile([128, NR * NQ * 128], DT, tag="fw")
            mt_t = cpool.tile([128, 3 * NJ * NJ * 128], DT, tag="mt")
            bal = _Balancer(nc)

            def inverse(Zg, go0, fgw):
                outt = iopool.tile([128, NJ, C, fgw], F32, tag=f"outt{fgw}",
                                   bufs=2)
                for jg in range(NJ):
                    psf = ppool.tile([128, C, 128], F32, tag="psC", bufs=2)
                    ps = psf[:, :, 0:fgw]
                    i = 0
                    for zg in range(3):
                        for rh in range(NJ):
                            lhsT = mt_t[
                                :,
                                ((zg * NJ + rh) * NJ + jg) * 128 :
                                ((zg * NJ + rh) * NJ + jg + 1) * 128,
                            ]
                            nc.tensor.matmul(
                                ps,
                                lhsT,
                                Zg[:, zg * NJ + rh, :, :],
                                start=(i == 0),
                                stop=(i == 3 * NJ - 1),
                            )
                            i += 1
                    nc.scalar.copy(out=outt[:, jg], in_=ps)
                nc.sync.dma_start(
                    out_d.ap()[:, :, :, go0 : go0 + fgw], outt[:]
                )

            Z = None
            zoff = 0  # next free g-slot in the current Z group
            zw = 0  # current Z group width
            zg0 = 0  # group's first g
            pending_inv = None  # (Z, g0, width) whose inverse is deferred
            for chunk, (g0, GH) in enumerate(CHUNKS):
                last = chunk == len(CHUNKS) - 1
                xin = iopool.tile([128, GH, U], DT, tag=f"xin{GH}", bufs=3)
                yin = iopool.tile([128, GH, W], DT, tag=f"yin{GH}", bufs=3)
                nc.sync.dma_start(xin[:], xp_d.ap()[:, g0 : g0 + GH, :])
                nc.sync.dma_start(yin[:], yp_d.ap()[:, g0 : g0 + GH, :])
                if chunk == 0:
                    # consts after the first input tiles: r-pieces in use order
                    for r in range(NR):
                        nc.sync.dma_start(
                            fw_t[:, r * NQ * 128 : (r + 1) * NQ * 128],
                            fw_d.ap()[:, r * NQ * 128 : (r + 1) * NQ * 128],
                        )
                if chunk == 1:
                    # mt is first needed by the inverse at the end of chunk 1
                    nc.sync.dma_start(mt_t[:], mt_d.ap())

                xs = spool.tile([128, NR, S, GH], DT, tag=f"xs{GH}", bufs=3)
                ys = spool.tile([128, NR, V, GH], DT, tag=f"ys{GH}", bufs=3)

                # ---- forward transforms (PE), spectra with g innermost ----
                for r in range(NR):
                    for cg in range(GH // GC):
                        ps = ppool.tile([128, S, GC], F32, tag="psA", bufs=4)
                        for qi, q in enumerate(QORDER):
                            sm = SMAX[q]
                            lhsT = fw_t[
                                :, (r * NQ + q) * 128 : (r * NQ + q + 1) * 128
                            ]
                            rhs = xin[
                                :,
                                cg * GC : (cg + 1) * GC,
                                q : q + BS * (sm - 1) + 1 : BS,
                            ].rearrange("p g s -> p s g")
                            nc.tensor.matmul(
                                ps[:, 0:sm, :],
                                lhsT,
                                rhs,
                                start=(qi == 0),
                                stop=(qi == NQ - 1),
                            )
                        nc.scalar.copy(
                            out=xs[:, r, :, cg * GC : (cg + 1) * GC], in_=ps[:]
                        )
                    psyf = ppool.tile([128, V, 64], F32, tag="psB", bufs=2)
                    psy = psyf[:, :, 0:GH]
                    for q in range(NJ):
                        lhsT = fw_t[:, (r * NQ + q) * 128 : (r * NQ + q + 1) * 128]
                        rhs = yin[:, :, q : q + BS * (V - 1) + 1 : BS].rearrange(
                            "p g v -> p v g"
                        )
                        nc.tensor.matmul(
                            psy, lhsT, rhs, start=(q == 0), stop=(q == NJ - 1)
                        )
                    nc.scalar.copy(out=ys[:, r], in_=psy)

                # inverse of the previous pair-group, emitted after this
                # chunk's fwd so the in-order PE queue never blocks on it
                if pending_inv is not None:
                    inverse(*pending_inv)
                    pending_inv = None

                # ---- pointwise products + v-sum add-tree (DVE/GpSimd) ----
                if Z is None:
                    zw = GH if last else GH + CHUNKS[chunk + 1][1]
                    zg0 = g0
                    zoff = 0
                    Z = zpool.tile([128, 3 * NJ, C, zw], DT, tag=f"Z{zw}")
                gsl = slice(zoff, zoff + GH)
                zoff += GH
                h = V // 2
                m = NJ * h * GH
                prs = []
                # all product pieces first (Pool-eligible, <=768 elems each);
                # tree ops after, DVE-only: DVE's in-order queue never waits
                # on a slow Pool op mid-chain
                for c in range(C):
                    pr1 = wpool.tile([128, NR, V, GH], DT, tag=f"pr1{GH}", bufs=2)
                    pr2 = wpool.tile([128, NR, V, GH], DT, tag=f"pr2{GH}", bufs=2)
                    prs.append((pr1, pr2))
                    for lo, hi in ((0, h), (h, V)):
                        bal.eng(m).tensor_mul(
                            pr1[:, 0:NJ, lo:hi, :],
                            xs[:, 0:NJ, c + lo : c + hi, :],
                            ys[:, 0:NJ, lo:hi, :],
                        )
                        bal.eng(m).tensor_mul(
                            pr1[:, NJ:NR, lo:hi, :],
                            xs[:, NJ:NR, c + lo : c + hi, :],
                            ys[:, NJ:NR, lo:hi, :],
                        )
                        bal.eng(m).tensor_mul(
                            pr2[:, 0:NJ, lo:hi, :],
                            xs[:, NJ:NR, c + lo : c + hi, :],
                            ys[:, 0:NJ, lo:hi, :],
                        )
                        bal.eng(m).tensor_mul(
                            pr2[:, NJ:NR, lo:hi, :],
                            xs[:, 0:NJ, c + lo : c + hi, :],
                            ys[:, NJ:NR, lo:hi, :],
                        )
                for c in range(C):
                    pr1, pr2 = prs[c]
                    t1 = wpool.tile([128, NR, V // 2, GH], DT, tag=f"t1{GH}", bufs=2)
                    bal.dve(NR * (V // 2) * GH).tensor_add(
                        t1[:], pr1[:, :, 0:V:2, :], pr1[:, :, 1:V:2, :]
                    )
                    t2 = wpool.tile([128, NR, V // 4, GH], DT, tag=f"t2{GH}", bufs=2)
                    bal.dve(NR * (V // 4) * GH).tensor_add(
                        t2[:], t1[:, :, 0 : V // 2 : 2, :], t1[:, :, 1 : V // 2 : 2, :]
                    )
                    t3 = wpool.tile([128, NR, GH], DT, tag=f"t3{GH}", bufs=2)
                    bal.dve(NR * GH).tensor_add(
                        t3[:], t2[:, :, 0, :], t2[:, :, 1, :]
                    )
                    bal.dve(NJ * GH).tensor_add(
                        Z[:, 0:NJ, c, gsl], t3[:, 0:NJ], t3[:, NJ:NR]
                    )
                    u1 = wpool.tile([128, NR, V // 2, GH], DT, tag=f"u1{GH}", bufs=2)
                    bal.dve(NR * (V // 2) * GH).tensor_add(
                        u1[:], pr2[:, :, 0:V:2, :], pr2[:, :, 1:V:2, :]
                    )
                    u2 = wpool.tile([128, NR, V // 4, GH], DT, tag=f"u2{GH}", bufs=2)
                    bal.dve(NR * (V // 4) * GH).tensor_add(
                        u2[:], u1[:, :, 0 : V // 2 : 2, :], u1[:, :, 1 : V // 2 : 2, :]
                    )
                    bal.dve(NR * GH).tensor_add(
                        Z[:, NJ : 3 * NJ, c, gsl], u2[:, :, 0, :], u2[:, :, 1, :]
                    )

                # ---- inverse transform for each completed pair-group ----
                if zoff == zw:
                    if last:
                        inverse(Z, zg0, zw)  # final group: nothing to defer behind
                    else:
                        pending_inv = (Z, zg0, zw)
                    Z = None

    nc.compile()
    return nc


def _prep_core_inputs(d1f, d2f, fw, mt, core):
    """d1f/d2f: [2048, 3000] fp32. Returns the in_map for `core`."""
    bf = ml_dtypes.bfloat16
    sl = slice(core * G, (core + 1) * G)
    xp = np.zeros((G, U * 128), dtype=np.float32)
    xp[:, SHIFT : SHIFT + NT] = d1f[sl]
    yp = np.zeros((G, W * 128), dtype=np.float32)
    yp[:, :NT] = d2f[sl]
    # device layouts: xpT[p, g, u] = xp[g, 128u + p]
    xpT = np.ascontiguousarray(xp.reshape(G, U, 128).transpose(2, 0, 1)).astype(bf)
    ypT = np.ascontiguousarray(yp.reshape(G, W, 128).transpose(2, 0, 1)).astype(bf)
    return {"xp": xpT, "yp": ypT, "fw": fw, "mt": mt}


def kernel(data1: np.ndarray, data2: np.ndarray) -> np.ndarray:
    import time

    d1f = np.ascontiguousarray(data1, dtype=np.float32).reshape(-1, NT)
    d2f = np.ascontiguousarray(data2, dtype=np.float32).reshape(-1, NT)
    fw, mt = _const_tiles()

    t0 = time.time()
    if "nc" not in _PE_CACHE:
        _PE_CACHE["nc"] = build_kernel()
    nc = _PE_CACHE["nc"]
    print(f"[kernel] build+compile {time.time() - t0:.1f}s", file=sys.stderr,
          flush=True)

    in_maps = [_prep_core_inputs(d1f, d2f, fw, mt, i) for i in range(NCORES)]
    t0 = time.time()
    res = run_bass_kernel_spmd(nc, in_maps, core_ids=list(range(NCORES)))
    print(f"[kernel] spmd run {time.time() - t0:.1f}s", file=sys.stderr, flush=True)
    global LAST_EXEC_NS, LAST_TRACE
    LAST_EXEC_NS = res.exec_time_ns
    LAST_TRACE = res.instructions_and_trace
    if res.exec_time_ns is not None:
        print(f"[kernel] HW exec {res.exec_time_ns} ns", file=sys.stderr, flush=True)

    outs = []
    for i in range(NCORES):
        o = np.asarray(res.results[i]["out"], dtype=np.float32)  # [128, NJ, C, G]
        # out[g, B*c + 128*jg + p] = o[p, jg, c, g]
        full = o.transpose(3, 2, 1, 0).reshape(G, C * B)
        outs.append(full[:, :LAGS])
    return np.concatenate(outs, axis=0).reshape(NB_PAIRS, NCH, LAGS)


# revision 24
# speedup vs baseline: 1.4229x; 1.0095x over previous
"""Trainium2 Bass kernel: batched time-domain cross-correlation.

Computes, for each of 2048 (=64x32) independent pairs (fp32):
    out[g, l] = sum_k d1[g, k + l - 301] * d2[g, k],   l in [0, 603)

Algorithm: overlap-save block correlation in a half-shift (negacyclic)
real-DFT basis; every matmul has a shared stationary operand (the
transform matrices) and batches all pairs in the moving operand:

  xp = d1 zero-padded/shifted; y = d2 zero-padded.
  out[B*c + j] = sum_v corr(w_{v+c}, y_v)[j]     (j in [0, B))
    w_s = xp[B*s : B*s + 2B]  (windows, stride B, length N=2B)
    y_v = y[B*v : B*v + B]    (blocks, zero-padded to N)
  Per-block circular corr via length-N negacyclic real DFT:
    bins k: Ur[k] = sum_n u[n] cos(pi n (2k+1)/N)
            Ui[k] = -sum_n u[n] sin(pi n (2k+1)/N),  k in [0, B)
    Z = X * conj(Y):  Zr = XrYr + XiYi ; Zia = XiYr ; Zib = XrYi
    z[0:B] = Mr@Zr + Mi@Zia - Mi@Zib  (exact: aliasing only corrupts j > B)

v2 layout: bf16 throughout (fp32 PSUM/output). Spectra stored with the
pair index g innermost ([128, NR, s, GH]); the v-sum of spectral
products runs as a pairwise add-tree on DVE/GpSimd (bf16 2x mode)
instead of tensor_reduce, with ops split across the two engines by a
greedy cost balancer.

Sharding: data-parallel over the 2048 pairs, 256 pairs per core, 8 cores.
"""

import sys

import numpy as np

if "/opt/trn_rl_repo" not in sys.path:  # harness safety; axon site usually set
    sys.path.insert(0, "/opt/trn_rl_repo")

import ml_dtypes

import concourse.bacc as bacc
import concourse.bass as bass
import concourse.mybir as mybir
import concourse.tile as tile
from concourse.bass_utils import run_bass_kernel_spmd

# ---- problem constants (hardcoded per contest contract) ----
NB_PAIRS, NCH, NT = 64, 32, 3000
LAGS = 603
SHIFT = 301  # NLAG + 1
NCORES = 8
G = (NB_PAIRS * NCH) // NCORES  # 256 pairs per core

B = 384  # lag/block granularity; N = 2B
N = 2 * B
V = 8  # ceil(NT / B) y blocks
C = 2  # ceil(LAGS / B) output lag blocks
S = V + C - 1  # 9 x windows
BS = B // 128  # 3 chunks per B
NQ = N // 128  # 6 contraction chunks of a window
NJ = B // 128  # 3 bins-half / out-j 128-groups
NR = 2 * NJ  # 6 spectral 128-groups
U = (S - 1) * BS + NQ  # 30 128-chunks in xp
W = V * BS  # 24 128-chunks in y
GH = 64  # pairs per chunk (4 chunks)
GC = 32  # x-fwd pair group (moving S*GC = 288)
# tapered chunk sizes: GH=56 lets the x-fwd psum tile use 504 of 512 psum
# slots (one matmul per (r,q) instead of two); small last chunk + its own
# inverse group keeps the drain short
CHUNKS = [(0, 56), (56, 56), (112, 56), (168, 56), (224, 32)]

# trailing zero chunks of xp: nonzero taps end at SHIFT+NT=3301 -> chunk 25.
# window s uses chunk 3s+q; window s contributes at position q iff 3s+q <= 25
SMAX = [min(S, (25 - q) // BS + 1) for q in range(NQ)]
QORDER = [1, 2, 3, 4, 5, 0]  # first/last emitted cover the full region

DT = mybir.dt.bfloat16
F32 = mybir.dt.float32

_PE_CACHE = {}
LAST_EXEC_NS = None
LAST_TRACE = None

# engine-balance cost constants (ns)
_DVE_PER_ELEM = 0.52
_DVE_FIXED = 85.0
_POOL_PER_ELEM = 1.98  # Pool gets no bf16 2x mode (DVE-only)
_POOL_FIXED = 130.0


class _Balancer:
    def __init__(self, nc):
        self.nc = nc
        self.tv = 0.0
        self.tg = 0.0

    def eng(self, elems):
        cv = elems * _DVE_PER_ELEM + _DVE_FIXED
        cg = elems * _POOL_PER_ELEM + _POOL_FIXED
        if max(self.tv + cv, self.tg) <= max(self.tv, self.tg + cg):
            self.tv += cv
            return self.nc.vector
        self.tg += cg
        return self.nc.gpsimd

    def dve(self, elems):
        self.tv += elems * _DVE_PER_ELEM + _DVE_FIXED
        return self.nc.vector


def _matrices():
    n = np.arange(N, dtype=np.float64)[:, None]
    k = np.arange(B, dtype=np.float64)[None, :]
    theta = np.pi * n * (2 * k + 1) / N
    ffull = np.concatenate([np.cos(theta), -np.sin(theta)], axis=1)  # [N, 2B]
    minv = np.linalg.inv(ffull.T)[:B, :]  # [B, 2B]
    return ffull.astype(np.float32), minv.astype(np.float32)


def _const_tiles():
    """FW [128, NR*NQ*128]: FW[i, ((r*NQ)+q)*128 + col] = Ffull[128q+i, 128r+col]
    MT [128, 3*NJ*NJ*128]: for zg in {Mr, Mi, -Mi}:
        MT[i, ((zg*NJ + rh)*NJ + jg)*128 + col] = M[128jg + col, 128rh + i]
    """
    ffull, minv = _matrices()
    fw = np.zeros((128, NR * NQ * 128), dtype=np.float32)
    for q in range(NQ):
        for r in range(NR):
            fw[:, (r * NQ + q) * 128 : (r * NQ + q + 1) * 128] = ffull[
                128 * q : 128 * (q + 1), 128 * r : 128 * (r + 1)
            ]
    mr = minv[:, :B]
    mi = minv[:, B:]
    # Karatsuba components: Z = [S1, S2, Su] with S1=sum XrYr, S2=sum XiYi,
    # Su=sum (Xi-Xr)(Yr+Yi);  Zr=S1+S2, Zi=Su+S1-S2
    # out = Mr@Zr + Mi@Zi = (Mr+Mi)@S1 + (Mr-Mi)@S2 + Mi@Su
    mats = [mr + mi, mr - mi, mi]
    mt = np.zeros((128, 3 * NJ * NJ * 128), dtype=np.float32)
    for zg in range(3):
        for rh in range(NJ):
            for jg in range(NJ):
                blk = mats[zg][128 * jg : 128 * (jg + 1), 128 * rh : 128 * (rh + 1)]
                base = ((zg * NJ + rh) * NJ + jg) * 128
                mt[:, base : base + 128] = blk.T
    bf = ml_dtypes.bfloat16
    return fw.astype(bf), mt.astype(bf)


def build_kernel():
    nc = bacc.Bacc(
        "TRN2",
        target_bir_lowering=False,
        debug=False,
        num_devices=NCORES,
    )

    xp_d = nc.dram_tensor("xp", [128, G, U], DT, kind="ExternalInput")
    yp_d = nc.dram_tensor("yp", [128, G, W], DT, kind="ExternalInput")
    fw_d = nc.dram_tensor("fw", [128, NR * NQ * 128], DT, kind="ExternalInput")
    mt_d = nc.dram_tensor("mt", [128, 3 * NJ * NJ * 128], DT, kind="ExternalInput")
    out_d = nc.dram_tensor("out", [128, NJ, C, G], F32, kind="ExternalOutput")

    with tile.TileContext(nc, trace_sim=False) as tc:
        with (
            tc.tile_pool(name="const", bufs=1) as cpool,
            tc.tile_pool(name="io", bufs=2) as iopool,
            tc.tile_pool(name="spec", bufs=2) as spool,
            tc.tile_pool(name="work", bufs=2) as wpool,
            tc.tile_pool(name="zpool", bufs=2) as zpool,
            tc.tile_pool(name="psum", bufs=1, space=bass.MemorySpace.PSUM) as ppool,
        ):
            fw_t = cpool.tile([128, NR * NQ * 128], DT, tag="fw")
            mt_t = cpool.tile([128, 3 * NJ * NJ * 128], DT, tag="mt")
            bal = _Balancer(nc)

            def inverse(Zg, go0, fgw):
                outt = iopool.tile([128, NJ, C, fgw], F32, tag=f"outt{fgw}",
                                   bufs=2)
                for jg in range(NJ):
                    psf = ppool.tile([128, C, 128], F32, tag="psC", bufs=2)
                    ps = psf[:, :, 0:fgw]
                    i = 0
                    for zg in range(3):
                        for rh in range(NJ):
                            lhsT = mt_t[
                                :,
                                ((zg * NJ + rh) * NJ + jg) * 128 :
                                ((zg * NJ + rh) * NJ + jg + 1) * 128,
                            ]
                            nc.tensor.matmul(
                                ps,
                                lhsT,
                                Zg[:, zg * NJ + rh, :, :],
                                start=(i == 0),
                                stop=(i == 3 * NJ - 1),
                            )
                            i += 1
                    nc.scalar.copy(out=outt[:, jg], in_=ps)
                nc.sync.dma_start(
                    out_d.ap()[:, :, :, go0 : go0 + fgw], outt[:]
                )

            Z = None
            zoff = 0  # next free g-slot in the current Z group
            zw = 0  # current Z group width
            zg0 = 0  # group's first g
            pending_inv = None  # (Z, g0, width) whose inverse is deferred
            for chunk, (g0, GH) in enumerate(CHUNKS):
                last = chunk == len(CHUNKS) - 1
                xin = iopool.tile([128, GH, U], DT, tag=f"xin{GH}", bufs=3)
                yin = iopool.tile([128, GH, W], DT, tag=f"yin{GH}", bufs=3)
                nc.sync.dma_start(xin[:], xp_d.ap()[:, g0 : g0 + GH, :])
                nc.sync.dma_start(yin[:], yp_d.ap()[:, g0 : g0 + GH, :])
                if chunk == 0:
                    # consts after the first input tiles: r-pieces in use order
                    for r in range(NR):
                        nc.sync.dma_start(
                            fw_t[:, r * NQ * 128 : (r + 1) * NQ * 128],
                            fw_d.ap()[:, r * NQ * 128 : (r + 1) * NQ * 128],
                        )
                if chunk == 1:
                    # mt is first needed by the inverse at the end of chunk 1
                    nc.sync.dma_start(mt_t[:], mt_d.ap())

                xs = spool.tile([128, NR, S, GH], DT, tag=f"xs{GH}", bufs=3)
                ys = spool.tile([128, NR, V, GH], DT, tag=f"ys{GH}", bufs=3)

                # ---- forward transforms (PE), spectra with g innermost ----
                for r in range(NR):
                    for cg in range(GH // GC):
                        ps = ppool.tile([128, S, GC], F32, tag="psA", bufs=4)
                        for qi, q in enumerate(QORDER):
                            sm = SMAX[q]
                            lhsT = fw_t[
                                :, (r * NQ + q) * 128 : (r * NQ + q + 1) * 128
                            ]
                            rhs = xin[
                                :,
                                cg * GC : (cg + 1) * GC,
                                q : q + BS * (sm - 1) + 1 : BS,
                            ].rearrange("p g s -> p s g")
                            nc.tensor.matmul(
                                ps[:, 0:sm, :],
                                lhsT,
                                rhs,
                                start=(qi == 0),
                                stop=(qi == NQ - 1),
                            )
                        nc.scalar.copy(
                            out=xs[:, r, :, cg * GC : (cg + 1) * GC], in_=ps[:]
                        )
                    psyf = ppool.tile([128, V, 64], F32, tag="psB", bufs=2)
                    psy = psyf[:, :, 0:GH]
                    for q in range(NJ):
                        lhsT = fw_t[:, (r * NQ + q) * 128 : (r * NQ + q + 1) * 128]
                        rhs = yin[:, :, q : q + BS * (V - 1) + 1 : BS].rearrange(
                            "p g v -> p v g"
                        )
                        nc.tensor.matmul(
                            psy, lhsT, rhs, start=(q == 0), stop=(q == NJ - 1)
                        )
                    nc.scalar.copy(out=ys[:, r], in_=psy)

                # inverse of the previous pair-group, emitted after this
                # chunk's fwd so the in-order PE queue never blocks on it
                if pending_inv is not None:
                    inverse(*pending_inv)
                    pending_inv = None

                # ---- pointwise products + v-sum add-tree (DVE/GpSimd) ----
                if Z is None:
                    zw = GH if last else GH + CHUNKS[chunk + 1][1]
                    zg0 = g0
                    zoff = 0
                    Z = zpool.tile([128, 3 * NJ, C, zw], DT, tag=f"Z{zw}")
                gsl = slice(zoff, zoff + GH)
                zoff += GH
                h = V // 2
                m = NJ * h * GH
                # Karatsuba prep (amortized over C): xd = Xi-Xr, ysum = Yr+Yi
                xd = wpool.tile([128, NJ, S, GH], DT, tag=f"xd{GH}", bufs=2)
                ysum = wpool.tile([128, NJ, V, GH], DT, tag=f"ysum{GH}", bufs=2)
                sh = (S + 1) // 2
                for lo, hi in ((0, sh), (sh, S)):
                    bal.eng(NJ * (hi - lo) * GH).tensor_sub(
                        xd[:, :, lo:hi, :],
                        xs[:, NJ:NR, lo:hi, :],
                        xs[:, 0:NJ, lo:hi, :],
                    )
                for lo, hi in ((0, h), (h, V)):
                    bal.eng(m).tensor_add(
                        ysum[:, :, lo:hi, :],
                        ys[:, 0:NJ, lo:hi, :],
                        ys[:, NJ:NR, lo:hi, :],
                    )
                prs = []
                # all product pieces first (Pool-eligible, <=768 elems each);
                # tree ops after, DVE-only: DVE's in-order queue never waits
                # on a slow Pool op mid-chain
                for c in range(C):
                    p12 = wpool.tile([128, NR, V, GH], DT, tag=f"p12{GH}", bufs=2)
                    uu = wpool.tile([128, NJ, V, GH], DT, tag=f"uu{GH}", bufs=2)
                    prs.append((p12, uu))
                    for lo, hi in ((0, h), (h, V)):
                        bal.eng(m).tensor_mul(
                            p12[:, 0:NJ, lo:hi, :],
                            xs[:, 0:NJ, c + lo : c + hi, :],
                            ys[:, 0:NJ, lo:hi, :],
                        )
                        bal.eng(m).tensor_mul(
                            p12[:, NJ:NR, lo:hi, :],
                            xs[:, NJ:NR, c + lo : c + hi, :],
                            ys[:, NJ:NR, lo:hi, :],
                        )
                        bal.eng(m).tensor_mul(
                            uu[:, :, lo:hi, :],
                            xd[:, :, c + lo : c + hi, :],
                            ysum[:, :, lo:hi, :],
                        )
                for c in range(C):
                    p12, uu = prs[c]
                    t1 = wpool.tile([128, NR, V // 2, GH], DT, tag=f"t1{GH}", bufs=2)
                    bal.dve(NR * (V // 2) * GH).tensor_add(
                        t1[:], p12[:, :, 0:V:2, :], p12[:, :, 1:V:2, :]
                    )
                    t2 = wpool.tile([128, NR, V // 4, GH], DT, tag=f"t2{GH}", bufs=2)
                    bal.dve(NR * (V // 4) * GH).tensor_add(
                        t2[:], t1[:, :, 0 : V // 2 : 2, :], t1[:, :, 1 : V // 2 : 2, :]
                    )
                    bal.dve(NR * GH).tensor_add(
                        Z[:, 0:NR, c, gsl], t2[:, :, 0, :], t2[:, :, 1, :]
                    )
                    w1 = wpool.tile([128, NJ, V // 2, GH], DT, tag=f"w1{GH}", bufs=2)
                    bal.dve(NJ * (V // 2) * GH).tensor_add(
                        w1[:], uu[:, :, 0:V:2, :], uu[:, :, 1:V:2, :]
                    )
                    w2 = wpool.tile([128, NJ, V // 4, GH], DT, tag=f"w2{GH}", bufs=2)
                    bal.dve(NJ * (V // 4) * GH).tensor_add(
                        w2[:], w1[:, :, 0 : V // 2 : 2, :], w1[:, :, 1 : V // 2 : 2, :]
                    )
                    bal.dve(NJ * GH).tensor_add(
                        Z[:, NR : 3 * NJ, c, gsl], w2[:, :, 0, :], w2[:, :, 1, :]
                    )

                # ---- inverse transform for each completed pair-group ----
                if zoff == zw:
                    if last:
                        inverse(Z, zg0, zw)  # final group: nothing to defer behind
                    else:
                        pending_inv = (Z, zg0, zw)
                    Z = None

    nc.compile()
    return nc


def _prep_core_inputs(d1f, d2f, fw, mt, core):
    """d1f/d2f: [2048, 3000] fp32. Returns the in_map for `core`."""
    bf = ml_dtypes.bfloat16
    sl = slice(core * G, (core + 1) * G)
    xp = np.zeros((G, U * 128), dtype=np.float32)
    xp[:, SHIFT : SHIFT + NT] = d1f[sl]
    yp = np.zeros((G, W * 128), dtype=np.float32)
    yp[:, :NT] = d2f[sl]
    # device layouts: xpT[p, g, u] = xp[g, 128u + p]
    xpT = np.ascontiguousarray(xp.reshape(G, U, 128).transpose(2, 0, 1)).astype(bf)
    ypT = np.ascontiguousarray(yp.reshape(G, W, 128).transpose(2, 0, 1)).astype(bf)
    return {"xp": xpT, "yp": ypT, "fw": fw, "mt": mt}


def kernel(data1: np.ndarray, data2: np.ndarray) -> np.ndarray:
    import time

    d1f = np.ascontiguousarray(data1, dtype=np.float32).reshape(-1, NT)
    d2f = np.ascontiguousarray(data2, dtype=np.float32).reshape(-1, NT)
    fw, mt = _const_tiles()

    t0 = time.time()
    if "nc" not in _PE_CACHE:
        _PE_CACHE["nc"] = build_kernel()
    nc = _PE_CACHE["nc"]
    print(f"[kernel] build+compile {time.time() - t0:.1f}s", file=sys.stderr,
          flush=True)

    in_maps = [_prep_core_inputs(d1f, d2f, fw, mt, i) for i in range(NCORES)]
    t0 = time.time()
    res = run_bass_kernel_spmd(nc, in_maps, core_ids=list(range(NCORES)))
    print(f"[kernel] spmd run {time.time() - t0:.1f}s", file=sys.stderr, flush=True)
    global LAST_EXEC_NS, LAST_TRACE
    LAST_EXEC_NS = res.exec_time_ns
    LAST_TRACE = res.instructions_and_trace
    if res.exec_time_ns is not None:
        print(f"[kernel] HW exec {res.exec_time_ns} ns", file=sys.stderr, flush=True)

    outs = []
    for i in range(NCORES):
        o = np.asarray(res.results[i]["out"], dtype=np.float32)  # [128, NJ, C, G]
        # out[g, B*c + 128*jg + p] = o[p, jg, c, g]
        full = o.transpose(3, 2, 1, 0).reshape(G, C * B)
        outs.append(full[:, :LAGS])
    return np.concatenate(outs, axis=0).reshape(NB_PAIRS, NCH, LAGS)
